# revision 24
# baseline (speedup 1.0000x reference)
"""Bahdanau attention TRN2 kernel.

B=8 batches data-parallel across 8 NeuronCores. Per core (one batch):
  q = query @ Wa -> qT [U=128part, Tq]   (PE, fp32)
  k = key @ Ua   -> kT [U=128part, Tv]
  X[u, j*Tq+i] = qT[u,i] + kT[u,j]       (DVE tensor_scalar, fp32 2x mode)
  t = tanh(X) -> fp16                    (ACT, the bottleneck ~225us)
  s[i, j] = sum_u scale[u]*t[u,i,j]      (PE: lhsT=t-block, rhs=scale[128,1])
  w = exp(s)  (no max subtraction; |s| <~ sum|scale| ~ 12, fp32-safe)
  wT = transpose(w) (PE), masked mul (DVE), Z = wT.T @ ones (PE)
  ctx = (wT.T @ value) / Z               (PE fp32 + DVE normalize)
Host side does layout-only prep: transposes of query/key, mask reshaping.
"""

import sys

if "/opt/trn_rl_repo" not in sys.path:
    sys.path.insert(0, "/opt/trn_rl_repo")

import numpy as np

import concourse.bacc as bacc
import concourse.bass as bass
import concourse.tile as tile
import concourse.mybir as mybir

F32 = mybir.dt.float32
F32R = mybir.dt.float32r
F16 = mybir.dt.float16
U8 = mybir.dt.uint8
AF = mybir.ActivationFunctionType

B, TQ, TV, D, U = 8, 512, 512, 512, 128
NJ = 16               # j's per tanh block
NBLK = TV // NJ
N_CORES = 8


import contextlib


def _maybe_for_i(tc, repeat):
    if repeat and repeat > 1:
        return tc.For_i(0, repeat, 1)
    return contextlib.nullcontext()


def _emit(nc, debug=False, skip_adds=False, skip_tanh=False, skip_smm=False,
          skip_tail=False, repeat_main=1):
    queryT = nc.dram_tensor("queryT", [D, TQ], F32R, kind="ExternalInput")
    keyT = nc.dram_tensor("keyT", [D, TV], F32R, kind="ExternalInput")
    value = nc.dram_tensor("value", [TV, D], F32R, kind="ExternalInput")
    wa = nc.dram_tensor("wa", [D, U], F32R, kind="ExternalInput")
    ua = nc.dram_tensor("ua", [D, U], F32R, kind="ExternalInput")
    scale = nc.dram_tensor("scale", [U], F32, kind="ExternalInput")
    maskrow = nc.dram_tensor("maskrow", [TV], F32, kind="ExternalInput")
    ident = nc.dram_tensor("ident", [128, 128], F32, kind="ExternalInput")
    ctx = nc.dram_tensor("ctx", [TQ, D], F32, kind="ExternalOutput")
    if debug:
        dbg_qT = nc.dram_tensor("dbg_qT", [U, TQ], F32, kind="ExternalOutput")
        dbg_kT = nc.dram_tensor("dbg_kT", [U, TV], F32, kind="ExternalOutput")
        dbg_w = nc.dram_tensor("dbg_w", [128, 4, TV], F32, kind="ExternalOutput")
        dbg_expT = nc.dram_tensor("dbg_expT", [128, 4, TQ], F32R, kind="ExternalOutput")
        dbg_zr = nc.dram_tensor("dbg_zr", [128, 4], F32, kind="ExternalOutput")

    with tile.TileContext(nc) as tc:
        with tc.tile_pool(name="const", bufs=1) as const:
            qT_sb = const.tile([U, TQ], F32, name="qT_sb")
            kT_sb = const.tile([U, TV], F32, name="kT_sb")
            scale_f32 = const.tile([U, 1], F32, name="scale_f32")
            scale_f16 = const.tile([U, 1], F16, name="scale_f16")
            maskbc_sb = const.tile([128, TV], F32, name="maskbc_sb")
            ident_sb = const.tile([128, 128], F32, name="ident_sb")
            value_sb = const.tile([128, 4, D], F32R, name="value_sb")
            w_sb = const.tile([128, 4, TV], F32, name="w_sb")
            wm_sb = const.tile([128, 4, TV], F32, name="wm_sb")
            expT_sb = const.tile([128, 4, TQ], F32R, name="expT_sb")
            z_sb = const.tile([128, 4], F32, name="z_sb")
            zr_sb = const.tile([128, 4], F32, name="zr_sb")
            octx_sb = const.tile([128, 4, D], F32, name="octx_sb")

            nc.sync.dma_start(out=scale_f32[:], in_=scale.ap()[:, None])
            nc.vector.tensor_copy(out=scale_f16[:], in_=scale_f32[:])
            mr = maskrow.ap()
            mask_bcast = bass.AP(tensor=mr.tensor, offset=mr.offset, ap=[[0, 128], [1, TV]])
            nc.sync.dma_start(out=maskbc_sb[:], in_=mask_bcast)
            nc.sync.dma_start(out=ident_sb[:], in_=ident.ap())
            for jc in range(4):
                nc.sync.dma_start(
                    out=value_sb[:, jc, :],
                    in_=value.ap()[jc * 128 : (jc + 1) * 128, :],
                )

            # ---- projections: qT = Wa^T @ queryT^T(chunks), kT likewise ----
            with (
                tc.tile_pool(name="projin", bufs=1) as projin,
                tc.tile_pool(name="projpsum", bufs=1, space="PSUM") as projpsum,
            ):
                qin = projin.tile([128, 4, TQ], F32R, name="qin")
                kin = projin.tile([128, 4, TV], F32R, name="kin")
                wa_sb = projin.tile([128, 4, U], F32R, name="wa_sb")
                ua_sb = projin.tile([128, 4, U], F32R, name="ua_sb")
                qT_r = queryT.ap().rearrange("(c p) i -> p c i", p=128)
                kT_r = keyT.ap().rearrange("(c p) i -> p c i", p=128)
                wa_r = wa.ap().rearrange("(c p) u -> p c u", p=128)
                ua_r = ua.ap().rearrange("(c p) u -> p c u", p=128)
                for dc in range(4):
                    nc.sync.dma_start(out=wa_sb[:, dc, :], in_=wa_r[:, dc, :])
                    nc.sync.dma_start(out=ua_sb[:, dc, :], in_=ua_r[:, dc, :])
                    nc.sync.dma_start(out=qin[:, dc, :], in_=qT_r[:, dc, :])
                    nc.sync.dma_start(out=kin[:, dc, :], in_=kT_r[:, dc, :])
                qT_ps = projpsum.tile([U, TQ], F32, name="qT_ps")
                kT_ps = projpsum.tile([U, TV], F32, name="kT_ps")
                for dc in range(4):
                    nc.tensor.matmul(
                        qT_ps[:], wa_sb[:, dc, :], qin[:, dc, :],
                        start=(dc == 0), stop=(dc == 3),
                    )
                for dc in range(4):
                    nc.tensor.matmul(
                        kT_ps[:], ua_sb[:, dc, :], kin[:, dc, :],
                        start=(dc == 0), stop=(dc == 3),
                    )
                nc.vector.tensor_copy(out=qT_sb[:], in_=qT_ps[:])
                nc.vector.tensor_copy(out=kT_sb[:], in_=kT_ps[:])

            # ---- main loop: adds (DVE) -> tanh (ACT) -> score matmuls (PE) ----
            with tc.tile_pool(name="spsum", bufs=1, space="PSUM") as spsum:
                s_ps = [
                    spsum.tile([128, TV], F32, name=f"s_ps{ic}") for ic in range(4)
                ]
                with (
                    tc.tile_pool(name="xpool", bufs=2) as xpool,
                    tc.tile_pool(name="tpool", bufs=2) as tpool,
                    _maybe_for_i(tc, repeat_main),
                ):
                    for blk in range(NBLK):
                        xt = xpool.tile([U, NJ * TQ], F32, name="xt", tag="x")
                        if not skip_adds:
                            for jj in range(NJ):
                                j = blk * NJ + jj
                                nc.vector.tensor_scalar_add(
                                    xt[:, jj * TQ : (jj + 1) * TQ],
                                    qT_sb[:],
                                    kT_sb[:, j : j + 1],
                                )
                        tt = tpool.tile([U, NJ * TQ], F16, name="tt", tag="t")
                        if not skip_tanh:
                            nc.scalar.activation(tt[:], xt[:], AF.Tanh)
                        if not skip_smm:
                            for jj in range(NJ):
                                j = blk * NJ + jj
                                for ic in range(4):
                                    nc.tensor.matmul(
                                        s_ps[ic][:, j : j + 1],
                                        tt[:, jj * TQ + ic * 128 : jj * TQ + (ic + 1) * 128],
                                        scale_f16[:],
                                    )
                # exp in [i, j] layout, PSUM -> SBUF (mask applied later)
                if not skip_tail:
                    for ic in range(4):
                        nc.scalar.activation(w_sb[:, ic, :], s_ps[ic][:], AF.Exp)

            if skip_tail:
                return
            # masked weights + row-sums Z on DVE: wm = w * mask, z = sum_j wm
            for ic in range(4):
                nc.vector.scalar_tensor_tensor(
                    out=wm_sb[:, ic, :],
                    in0=w_sb[:, ic, :],
                    scalar=1.0,
                    in1=maskbc_sb[:],
                    op0=mybir.AluOpType.mult,
                    op1=mybir.AluOpType.mult,
                    accum_out=z_sb[:, ic : ic + 1],
                )
            nc.vector.reciprocal(out=zr_sb[:], in_=z_sb[:])

            # ---- tail: transpose, context, normalize ----
            with (
                tc.tile_pool(name="tailpsum", bufs=1, space="PSUM") as tailpsum,
                tc.tile_pool(name="wtpsum", bufs=2, space="PSUM") as wtpsum,
            ):
                c_ps = [
                    tailpsum.tile([128, D], F32, name=f"c_ps{ic}") for ic in range(4)
                ]
                for jc in range(4):
                    wT_ps = wtpsum.tile([128, TQ], F32, name="wT_ps", tag="wt")
                    for ic in range(4):
                        nc.tensor.transpose(
                            wT_ps[:, ic * 128 : (ic + 1) * 128],
                            wm_sb[:, ic, jc * 128 : (jc + 1) * 128],
                            ident_sb[:],
                        )
                    nc.vector.tensor_copy(out=expT_sb[:, jc, :], in_=wT_ps[:])
                    for ic in range(4):
                        nc.tensor.matmul(
                            c_ps[ic][:],
                            expT_sb[:, jc, ic * 128 : (ic + 1) * 128],
                            value_sb[:, jc, :],
                            start=(jc == 0), stop=(jc == 3),
                        )
                for ic in range(4):
                    nc.vector.tensor_scalar_mul(
                        out=octx_sb[:, ic, :], in0=c_ps[ic][:], scalar1=zr_sb[:, ic : ic + 1]
                    )
                    nc.sync.dma_start(
                        out=ctx.ap()[ic * 128 : (ic + 1) * 128, :],
                        in_=octx_sb[:, ic, :],
                    )
                if debug:
                    nc.sync.dma_start(out=dbg_qT.ap(), in_=qT_sb[:])
                    nc.sync.dma_start(out=dbg_kT.ap(), in_=kT_sb[:])
                    nc.sync.dma_start(out=dbg_w.ap(), in_=w_sb[:])
                    nc.sync.dma_start(out=dbg_expT.ap(), in_=expT_sb[:])
                    nc.sync.dma_start(out=dbg_zr.ap(), in_=zr_sb[:])


class _Runner:
    """Builds the Bass module once and holds a reusable jitted shard_map
    callable (mirrors concourse.bass2jax.run_bass_via_pjrt, but persistent
    so repeat calls don't re-jit/re-compile)."""

    def __init__(self, debug=False):
        import jax
        from concourse.bass2jax import install_neuronx_cc_hook, _bass_exec_p
        from jax.experimental.shard_map import shard_map
        from jax.sharding import Mesh, PartitionSpec

        self.jax = jax
        nc = bacc.Bacc(
            "TRN2", target_bir_lowering=False, debug=False,
            enable_asserts=False, num_devices=N_CORES,
            enable_partition_id=False,
        )
        _emit(nc, debug=debug)
        nc.compile()
        self.nc = nc

        install_neuronx_cc_hook()
        in_names, out_names, out_avals = [], [], []
        for alloc in nc.m.functions[0].allocations:
            if not isinstance(alloc, mybir.MemoryLocationSet):
                continue
            name = alloc.memorylocations[0].name
            if alloc.kind == "ExternalInput":
                in_names.append(name)
            elif alloc.kind == "ExternalOutput":
                out_names.append(name)
                out_avals.append(
                    jax.core.ShapedArray(
                        tuple(alloc.tensor_shape), mybir.dt.np(alloc.dtype)
                    )
                )
        assert nc.partition_id_tensor is None
        self.in_names = in_names
        self.out_names = out_names
        self.out_avals = out_avals
        n_params = len(in_names)
        n_outs = len(out_names)
        all_names = tuple(in_names + out_names)

        def _body(*args):
            outs = _bass_exec_p.bind(
                *args,
                out_avals=tuple(out_avals),
                in_names=all_names,
                out_names=tuple(out_names),
                lowering_input_output_aliases=(),
                sim_require_finite=True,
                sim_require_nnan=True,
                nc=nc,
            )
            return tuple(outs)

        devices = jax.devices()[:N_CORES]
        self.mesh = Mesh(np.asarray(devices), ("core",))
        self.pspec = PartitionSpec("core")
        in_specs = (self.pspec,) * (n_params + n_outs)
        out_specs = (self.pspec,) * n_outs
        donate = tuple(range(n_params, n_params + n_outs))
        self.sharded = jax.jit(
            shard_map(
                _body, mesh=self.mesh, in_specs=in_specs, out_specs=out_specs,
                check_rep=False,
            ),
            donate_argnums=donate,
            keep_unused=True,
        )

    def concat_inputs(self, in_maps):
        return [
            np.concatenate([np.asarray(m[name]) for m in in_maps], axis=0)
            for name in self.in_names
        ]

    def fresh_zeros(self):
        return [
            np.zeros((N_CORES * a.shape[0], *a.shape[1:]), a.dtype)
            for a in self.out_avals
        ]

    def run(self, in_maps):
        out_arrs = self.sharded(*self.concat_inputs(in_maps), *self.fresh_zeros())
        a = self.out_avals[0]
        return np.asarray(out_arrs[0]).reshape(N_CORES, *a.shape)

    def run_all(self, in_maps):
        out_arrs = self.sharded(*self.concat_inputs(in_maps), *self.fresh_zeros())
        return {
            name: np.asarray(out_arrs[i]).reshape(
                N_CORES, *self.out_avals[i].shape
            )
            for i, name in enumerate(self.out_names)
        }


_runner = None


def _get_runner():
    global _runner
    if _runner is None:
        _runner = _Runner()
    return _runner


def _make_in_maps(query, key, value, mask, Wa, Ua, scale):
    query = np.asarray(query, dtype=np.float32)
    key = np.asarray(key, dtype=np.float32)
    value = np.asarray(value, dtype=np.float32)
    mask = np.asarray(mask)
    Wa = np.ascontiguousarray(np.asarray(Wa, dtype=np.float32))
    Ua = np.ascontiguousarray(np.asarray(Ua, dtype=np.float32))
    scale = np.ascontiguousarray(np.asarray(scale, dtype=np.float32))
    ident = np.eye(128, dtype=np.float32)
    in_maps = []
    for b in range(B):
        in_maps.append(
            {
                "queryT": np.ascontiguousarray(query[b].T),
                "keyT": np.ascontiguousarray(key[b].T),
                "value": np.ascontiguousarray(value[b]),
                "wa": Wa,
                "ua": Ua,
                "scale": scale,
                "maskrow": mask[b].astype(np.float32),
                "ident": ident,
            }
        )
    return in_maps


def kernel(query, key, value, mask, Wa, Ua, scale):
    r = _get_runner()
    in_maps = _make_in_maps(query, key, value, mask, Wa, Ua, scale)
    return r.run(in_maps)
